# revision 1
# baseline (speedup 1.0000x reference)
"""Trainium2 Bass kernel for the MOLLI T1-fitting unrolled-GD problem.

The reference computation reduces to (the denoiser CNN output is dead code,
and x - range_constraint(x) == 0 since x is always pre-clipped):

    x0 = clip([max_q|b|, 2*max_q|b|, 1], 0, [3,6,50])
    300 GD steps with lr=2.0, then 10 steps with lr=mu[i], where per pixel:
        e_q  = exp(-tau_q * R1)
        r_q  = A - B*e_q - b_q
        gA   = mean_q r_q ; gB = -mean_q r_q e_q ; gR1 = B*mean_q tau_q r_q e_q
        x    = clip(x - lr*g, 0, [3,6,50])
    output = stack([x0, x_after_300, ..., x_after_310])  # [12,8,192,192,3]

Sharding: pure data parallel, one batch image per NeuronCore (8 cores).
Per core: 36864 pixels laid out as SBUF [128 partitions, 288 free]; the
11-echo slab is [128, 11*288]. Everything stays SBUF-resident for all 310
iterations; only the 12 snapshots are DMAed out.
"""

import numpy as np
from contextlib import ExitStack

NQ = 11
NPIX = 192 * 192          # 36864 pixels per image/core
P = 128                   # SBUF partitions
FD = NPIX // P            # 288 free elements per partition
N_GD = 300
NS = 10
N_CORES = 8
LIM_A, LIM_B, LIM_R = 3.0, 6.0, 50.0
VARIANT = "v11"


def _build_nc(tau, lrs, snap_iters, compute_fp16=False, variant="v2"):
    """Build the Bass program. tau: list of 11 floats. lrs: list of 310
    learning rates. snap_iters: set of iteration indices after which to
    snapshot (0-indexed).

    Variants (HW-measured per 8-core run, axon-null-subtracted):
      v1  small per-q ops                              ~8.1 ms
      v2  big slab ops (broadcast APs) + DVE add trees ~7.9 ms
      v3  v2 + two trees on GpSimd                     ~10.2 ms (port clash)
      v4  tau-weighting on ACT, chunked exp/product/ste
          interleave, R1-first update order            ~6.3-6.9 ms
      v5  v4 + all three sums as fused triple trees    ~8.4 ms (ACT barrier)
      v6  v4 + Sr/Sre pair-fused tree                  ~7.1 ms
      v7  v4 with three product chunks                 ~= v4
      v8  v4 + all 11 exps hoisted to iteration start
          (removes in-order-ACT stall on chunk-2 E)    v4 - ~5%  <- used
      v9  v8 with (4,7) chunks                         worse
      v11 v8 with one full-slab product chunk
          (4 product ops; viable once exps hoisted)    v8 - ~3%  <- used
    """
    import concourse.bass as bass
    import concourse.tile as tile
    import concourse.mybir as mybir
    from concourse import bacc

    f32 = mybir.dt.float32
    f16 = mybir.dt.float16
    cdt = f16 if compute_fp16 else f32
    Alu = mybir.AluOpType
    Act = mybir.ActivationFunctionType

    n_iters = len(lrs)
    n_snaps = 1 + len(snap_iters)

    nc = bacc.Bacc(trn_type="TRN2", target_bir_lowering=False, debug=False)
    b_d = nc.dram_tensor("b", [P, NQ * FD], cdt, kind="ExternalInput").ap()
    out_d = nc.dram_tensor(
        "out", [n_snaps, P, 3 * FD], f32, kind="ExternalOutput"
    ).ap()

    with ExitStack() as ctx:
        tc = ctx.enter_context(tile.TileContext(nc))
        state = ctx.enter_context(tc.tile_pool(name="state", bufs=1))
        scratch = ctx.enter_context(tc.tile_pool(name="scratch", bufs=3))

        bsb = state.tile([P, NQ * FD], cdt, tag="bsb")
        E = state.tile([P, NQ * FD], cdt, tag="E")
        A = state.tile([P, FD], f32, tag="A")
        B = state.tile([P, FD], f32, tag="B")
        R1 = state.tile([P, FD], f32, tag="R1")
        Sr = state.tile([P, FD], cdt, tag="Sr")
        Sre = state.tile([P, FD], cdt, tag="Sre")
        Stre = state.tile([P, FD], cdt, tag="Stre")

        def blk(t, q):
            return t[:, q * FD:(q + 1) * FD]

        # Load b (one DMA; ~1.6MB)
        nc.sync.dma_start(bsb[:], b_d[:])

        # ---- init: A0 = max_q |b_q| ----
        absb = scratch.tile([P, NQ * FD], cdt, tag="absb")
        nc.scalar.activation(absb[:], bsb[:], Act.Abs)
        t1 = scratch.tile([P, 5 * FD], cdt, tag="i5")
        nc.vector.tensor_tensor(
            t1[:], absb[:, 0:5 * FD], absb[:, 5 * FD:10 * FD], Alu.max)
        t2 = scratch.tile([P, 2 * FD], cdt, tag="i2")
        nc.vector.tensor_tensor(
            t2[:], t1[:, 0:2 * FD], t1[:, 2 * FD:4 * FD], Alu.max)
        t3 = scratch.tile([P, FD], cdt, tag="i1")
        nc.vector.tensor_tensor(t3[:], t2[:, 0:FD], t2[:, FD:2 * FD], Alu.max)
        nc.vector.tensor_tensor(t3[:], t3[:], t1[:, 4 * FD:5 * FD], Alu.max)
        nc.vector.tensor_tensor(t3[:], t3[:], blk(absb, 10), Alu.max)
        # A = clip(A0,0,3) ; B = min(2*A0, 6) ; R1 = 1
        nc.vector.tensor_scalar(A[:], t3[:], 0.0, LIM_A, Alu.max, Alu.min)
        nc.vector.tensor_scalar(B[:], t3[:], 2.0, LIM_B, Alu.mult, Alu.min)
        nc.vector.memset(R1[:], 1.0)

        def snapshot(s):
            nc.sync.dma_start(out_d[s, :, 0:FD], A[:])
            nc.sync.dma_start(out_d[s, :, FD:2 * FD], B[:])
            nc.sync.dma_start(out_d[s, :, 2 * FD:3 * FD], R1[:])

        snapshot(0)

        # persistent slabs for the big-op forms
        if variant == "v5":
            # one contiguous slab holding [r | re | tau*re], fused tree temps
            prod = state.tile([P, 3 * NQ * FD], cdt, tag="prod")
            tr5t = state.tile([P, 3 * 5 * FD], cdt, tag="tr5t")
            tr2t = state.tile([P, 3 * 2 * FD], cdt, tag="tr2t")
            sums3 = state.tile([P, 3 * FD], cdt, tag="sums3")
        elif variant in ("v6", "v12"):
            # [r | re] contiguous for pair-fused trees; ste separate
            prod = state.tile([P, 2 * NQ * FD], cdt, tag="prod")
            ste = state.tile([P, NQ * FD], cdt, tag="ste")
            tr5t = state.tile([P, 2 * 5 * FD], cdt, tag="tr5t")
            tr2t = state.tile([P, 2 * 2 * FD], cdt, tag="tr2t")
            sums2 = state.tile([P, 2 * FD], cdt, tag="sums2")
            trees = {"c": (
                state.tile([P, 5 * FD], cdt, tag="tr5c", name="tr5c"),
                state.tile([P, 2 * FD], cdt, tag="tr2c", name="tr2c"))}
        elif variant != "v1":
            rsl = state.tile([P, NQ * FD], cdt, tag="rsl")
            resl = state.tile([P, NQ * FD], cdt, tag="resl")
            ste = state.tile([P, NQ * FD], cdt, tag="ste")
            trees = {}
            for nm in ("a", "b", "c"):
                trees[nm] = (
                    state.tile([P, 5 * FD], cdt, tag="tr5" + nm,
                               name="tr5" + nm),
                    state.tile([P, 2 * FD], cdt, tag="tr2" + nm,
                               name="tr2" + nm))

        def tree_sum(dst, slab, eng, nm):
            # dst [P,FD] = sum of the 11 FD-blocks of slab
            tr5, tr2 = trees[nm]
            eng.tensor_add(tr5[:], slab[:, 0:5 * FD], slab[:, 5 * FD:10 * FD])
            eng.tensor_add(tr2[:], tr5[:, 0:2 * FD], tr5[:, 2 * FD:4 * FD])
            eng.tensor_add(dst[:], tr2[:, 0:FD], tr2[:, FD:2 * FD])
            eng.tensor_add(dst[:], dst[:], tr5[:, 4 * FD:5 * FD])
            eng.tensor_add(dst[:], dst[:], slab[:, 10 * FD:11 * FD])

        snap_idx = 1
        for i in range(n_iters):
            c1 = float(lrs[i]) / NQ
            if variant == "v1":
                for q in range(NQ):
                    eq = blk(E, q)
                    nc.scalar.activation(
                        eq, R1[:], Act.Exp, scale=-float(tau[q]))
                    tmp = scratch.tile([P, FD], cdt, tag="tmp")
                    nc.vector.tensor_mul(tmp[:], B[:], eq)
                    u = scratch.tile([P, FD], cdt, tag="u")
                    nc.vector.tensor_sub(u[:], A[:], blk(bsb, q))
                    if q == 0:
                        nc.vector.tensor_sub(Sr[:], u[:], tmp[:])
                        nc.vector.tensor_mul(Sre[:], Sr[:], eq)
                        nc.vector.tensor_scalar_mul(
                            Stre[:], Sre[:], float(tau[q]))
                    else:
                        r = scratch.tile([P, FD], cdt, tag="r")
                        nc.vector.tensor_sub(r[:], u[:], tmp[:])
                        nc.vector.tensor_add(Sr[:], Sr[:], r[:])
                        re = scratch.tile([P, FD], cdt, tag="re")
                        nc.vector.tensor_mul(re[:], r[:], eq)
                        nc.vector.tensor_add(Sre[:], Sre[:], re[:])
                        nc.vector.scalar_tensor_tensor(
                            Stre[:], re[:], float(tau[q]), Stre[:],
                            Alu.mult, Alu.add)
            elif variant == "v5":
                # like v4, but rsl/resl/ste live in one contiguous slab and
                # the three 11-block sums run as fused triple-width trees
                prod4 = prod[:].rearrange("p (s q f) -> p s q f", s=3, q=NQ)
                for lo, hi in ((0, 6), (6, NQ)):
                    s = slice(lo * FD, hi * FD)
                    nq = hi - lo

                    def p3(sl):
                        return prod[:, sl * NQ * FD + lo * FD:
                                    sl * NQ * FD + hi * FD] \
                            .rearrange("p (q f) -> p q f", q=nq)

                    Abc = A[:].unsqueeze(1).broadcast_to([P, nq, FD])
                    Bbc = B[:].unsqueeze(1).broadcast_to([P, nq, FD])
                    e3 = E[:, s].rearrange("p (q f) -> p q f", q=nq)
                    bb3 = bsb[:, s].rearrange("p (q f) -> p q f", q=nq)
                    for q in range(lo, hi):
                        nc.scalar.activation(
                            blk(E, q), R1[:], Act.Exp, scale=-float(tau[q]))
                    nc.vector.tensor_tensor(p3(0), Abc, bb3, Alu.subtract)
                    nc.vector.tensor_tensor(p3(1), Bbc, e3, Alu.mult)
                    nc.vector.tensor_tensor(p3(0), p3(0), p3(1), Alu.subtract)
                    nc.vector.tensor_tensor(p3(1), p3(0), e3, Alu.mult)
                    for q in range(lo, hi):
                        nc.scalar.activation(
                            prod[:, (2 * NQ + q) * FD:(2 * NQ + q + 1) * FD],
                            prod[:, (NQ + q) * FD:(NQ + q + 1) * FD],
                            Act.Copy, scale=float(tau[q]))
                # fused triple trees: sums3[:, s, :] = sum_q prod4[:, s, q, :]
                tr5t4 = tr5t[:].rearrange("p (s k f) -> p s k f", s=3, k=5)
                tr2t4 = tr2t[:].rearrange("p (s k f) -> p s k f", s=3, k=2)
                nc.vector.tensor_tensor(
                    tr5t4, prod4[:, :, 0:5, :], prod4[:, :, 5:10, :], Alu.add)
                nc.vector.tensor_tensor(
                    tr2t4, tr5t4[:, :, 0:2, :], tr5t4[:, :, 2:4, :], Alu.add)
                s3 = sums3[:].rearrange("p (s f) -> p s f", s=3)
                nc.vector.tensor_tensor(
                    s3, tr2t4[:, :, 0, :], tr2t4[:, :, 1, :], Alu.add)
                nc.vector.tensor_tensor(s3, s3, tr5t4[:, :, 4, :], Alu.add)
                nc.vector.tensor_tensor(s3, s3, prod4[:, :, 10, :], Alu.add)
                SrV = sums3[:, 0 * FD:1 * FD]
                SreV = sums3[:, 1 * FD:2 * FD]
                StreV = sums3[:, 2 * FD:3 * FD]
                t = scratch.tile([P, FD], f32, tag="t")
                nc.vector.scalar_tensor_tensor(
                    t[:], StreV, -c1, B[:], Alu.mult, Alu.mult)
                nc.vector.tensor_add(R1[:], R1[:], t[:])
                nc.vector.tensor_scalar(
                    R1[:], R1[:], 0.0, LIM_R, Alu.max, Alu.min)
                nc.vector.scalar_tensor_tensor(
                    A[:], SrV, -c1, A[:], Alu.mult, Alu.add)
                nc.vector.tensor_scalar(
                    A[:], A[:], 0.0, LIM_A, Alu.max, Alu.min)
                nc.vector.scalar_tensor_tensor(
                    B[:], SreV, c1, B[:], Alu.mult, Alu.add)
                nc.vector.tensor_scalar(
                    B[:], B[:], 0.0, LIM_B, Alu.max, Alu.min)
                if i in snap_iters:
                    snapshot(snap_idx)
                    snap_idx += 1
                continue
            elif variant == "v12":
                # v11 (hoisted exps, full-slab products) + pair-fused Sr/Sre
                for q in range(NQ):
                    nc.scalar.activation(
                        blk(E, q), R1[:], Act.Exp, scale=-float(tau[q]))
                rslv = prod[:, 0:NQ * FD].rearrange(
                    "p (q f) -> p q f", q=NQ)
                reslv = prod[:, NQ * FD:2 * NQ * FD].rearrange(
                    "p (q f) -> p q f", q=NQ)
                Ev = E[:].rearrange("p (q f) -> p q f", q=NQ)
                bbv = bsb[:].rearrange("p (q f) -> p q f", q=NQ)
                Abc = A[:].unsqueeze(1).broadcast_to([P, NQ, FD])
                Bbc = B[:].unsqueeze(1).broadcast_to([P, NQ, FD])
                nc.vector.tensor_tensor(rslv, Abc, bbv, Alu.subtract)
                nc.vector.tensor_tensor(reslv, Bbc, Ev, Alu.mult)
                nc.vector.tensor_tensor(rslv, rslv, reslv, Alu.subtract)
                nc.vector.tensor_tensor(reslv, rslv, Ev, Alu.mult)
                for q in range(NQ):
                    nc.scalar.activation(
                        blk(ste, q),
                        prod[:, (NQ + q) * FD:(NQ + q + 1) * FD],
                        Act.Copy, scale=float(tau[q]))
                tree_sum(Stre, ste, nc.vector, "c")
                t = scratch.tile([P, FD], f32, tag="t")
                nc.vector.scalar_tensor_tensor(
                    t[:], Stre[:], -c1, B[:], Alu.mult, Alu.mult)
                nc.vector.tensor_add(R1[:], R1[:], t[:])
                nc.vector.tensor_scalar(
                    R1[:], R1[:], 0.0, LIM_R, Alu.max, Alu.min)
                prod4 = prod[:].rearrange("p (s q f) -> p s q f", s=2, q=NQ)
                tr5t4 = tr5t[:].rearrange("p (s k f) -> p s k f", s=2, k=5)
                tr2t4 = tr2t[:].rearrange("p (s k f) -> p s k f", s=2, k=2)
                nc.vector.tensor_tensor(
                    tr5t4, prod4[:, :, 0:5, :], prod4[:, :, 5:10, :], Alu.add)
                nc.vector.tensor_tensor(
                    tr2t4, tr5t4[:, :, 0:2, :], tr5t4[:, :, 2:4, :], Alu.add)
                s2v = sums2[:].rearrange("p (s f) -> p s f", s=2)
                nc.vector.tensor_tensor(
                    s2v, tr2t4[:, :, 0, :], tr2t4[:, :, 1, :], Alu.add)
                nc.vector.tensor_tensor(s2v, s2v, tr5t4[:, :, 4, :], Alu.add)
                nc.vector.tensor_tensor(s2v, s2v, prod4[:, :, 10, :], Alu.add)
                nc.vector.scalar_tensor_tensor(
                    A[:], sums2[:, 0:FD], -c1, A[:], Alu.mult, Alu.add)
                nc.vector.tensor_scalar(
                    A[:], A[:], 0.0, LIM_A, Alu.max, Alu.min)
                nc.vector.scalar_tensor_tensor(
                    B[:], sums2[:, FD:2 * FD], c1, B[:], Alu.mult, Alu.add)
                nc.vector.tensor_scalar(
                    B[:], B[:], 0.0, LIM_B, Alu.max, Alu.min)
                if i in snap_iters:
                    snapshot(snap_idx)
                    snap_idx += 1
                continue
            elif variant == "v6":
                # v4 with the Sr/Sre trees fused as one double-width tree
                for lo, hi in ((0, 6), (6, NQ)):
                    s = slice(lo * FD, hi * FD)
                    nq = hi - lo

                    def p2(sl):
                        return prod[:, sl * NQ * FD + lo * FD:
                                    sl * NQ * FD + hi * FD] \
                            .rearrange("p (q f) -> p q f", q=nq)

                    Abc = A[:].unsqueeze(1).broadcast_to([P, nq, FD])
                    Bbc = B[:].unsqueeze(1).broadcast_to([P, nq, FD])
                    e3 = E[:, s].rearrange("p (q f) -> p q f", q=nq)
                    bb3 = bsb[:, s].rearrange("p (q f) -> p q f", q=nq)
                    for q in range(lo, hi):
                        nc.scalar.activation(
                            blk(E, q), R1[:], Act.Exp, scale=-float(tau[q]))
                    nc.vector.tensor_tensor(p2(0), Abc, bb3, Alu.subtract)
                    nc.vector.tensor_tensor(p2(1), Bbc, e3, Alu.mult)
                    nc.vector.tensor_tensor(p2(0), p2(0), p2(1), Alu.subtract)
                    nc.vector.tensor_tensor(p2(1), p2(0), e3, Alu.mult)
                    for q in range(lo, hi):
                        nc.scalar.activation(
                            blk(ste, q),
                            prod[:, (NQ + q) * FD:(NQ + q + 1) * FD],
                            Act.Copy, scale=float(tau[q]))
                tree_sum(Stre, ste, nc.vector, "c")
                t = scratch.tile([P, FD], f32, tag="t")
                nc.vector.scalar_tensor_tensor(
                    t[:], Stre[:], -c1, B[:], Alu.mult, Alu.mult)
                nc.vector.tensor_add(R1[:], R1[:], t[:])
                nc.vector.tensor_scalar(
                    R1[:], R1[:], 0.0, LIM_R, Alu.max, Alu.min)
                # pair-fused Sr/Sre tree over [r | re]
                prod4 = prod[:].rearrange("p (s q f) -> p s q f", s=2, q=NQ)
                tr5t4 = tr5t[:].rearrange("p (s k f) -> p s k f", s=2, k=5)
                tr2t4 = tr2t[:].rearrange("p (s k f) -> p s k f", s=2, k=2)
                nc.vector.tensor_tensor(
                    tr5t4, prod4[:, :, 0:5, :], prod4[:, :, 5:10, :], Alu.add)
                nc.vector.tensor_tensor(
                    tr2t4, tr5t4[:, :, 0:2, :], tr5t4[:, :, 2:4, :], Alu.add)
                s2v = sums2[:].rearrange("p (s f) -> p s f", s=2)
                nc.vector.tensor_tensor(
                    s2v, tr2t4[:, :, 0, :], tr2t4[:, :, 1, :], Alu.add)
                nc.vector.tensor_tensor(s2v, s2v, tr5t4[:, :, 4, :], Alu.add)
                nc.vector.tensor_tensor(s2v, s2v, prod4[:, :, 10, :], Alu.add)
                nc.vector.scalar_tensor_tensor(
                    A[:], sums2[:, 0:FD], -c1, A[:], Alu.mult, Alu.add)
                nc.vector.tensor_scalar(
                    A[:], A[:], 0.0, LIM_A, Alu.max, Alu.min)
                nc.vector.scalar_tensor_tensor(
                    B[:], sums2[:, FD:2 * FD], c1, B[:], Alu.mult, Alu.add)
                nc.vector.tensor_scalar(
                    B[:], B[:], 0.0, LIM_B, Alu.max, Alu.min)
                if i in snap_iters:
                    snapshot(snap_idx)
                    snap_idx += 1
                continue
            elif variant in ("v4", "v7", "v8", "v9", "v11", "v13"):
                # products per chunk; ste (ACT) interleaved. v4/v7 emit exps
                # per chunk; v8/v9 hoist all 11 exps to the iteration start
                # (ACT runs its stream in order — per-chunk emission makes
                # chunk 2's exps wait behind chunk 1's ste copies, which
                # wait on re-chunk-1, stalling DVE's tmp-chunk-2).
                if variant == "v7":
                    chunks = ((0, 4), (4, 8), (8, NQ))
                elif variant == "v9":
                    chunks = ((0, 4), (4, NQ))
                elif variant in ("v11", "v13"):
                    chunks = ((0, NQ),)
                else:
                    chunks = ((0, 6), (6, NQ))
                if variant in ("v8", "v9", "v11", "v13"):
                    for q in range(NQ):
                        nc.scalar.activation(
                            blk(E, q), R1[:], Act.Exp, scale=-float(tau[q]))
                for lo, hi in chunks:
                    s = slice(lo * FD, hi * FD)
                    nq = hi - lo
                    def b3(x):
                        return x[:, s].rearrange("p (q f) -> p q f", q=nq)
                    Abc = A[:].unsqueeze(1).broadcast_to([P, nq, FD])
                    Bbc = B[:].unsqueeze(1).broadcast_to([P, nq, FD])
                    if variant in ("v4", "v7"):
                        for q in range(lo, hi):
                            nc.scalar.activation(
                                blk(E, q), R1[:], Act.Exp,
                                scale=-float(tau[q]))
                    nc.vector.tensor_tensor(
                        b3(rsl), Abc, b3(bsb), Alu.subtract)
                    nc.vector.tensor_tensor(
                        b3(resl), Bbc, b3(E), Alu.mult)
                    if variant == "v13":
                        # all-contiguous operands: flat 2-D APs
                        nc.vector.tensor_tensor(
                            rsl[:, s], rsl[:, s], resl[:, s], Alu.subtract)
                        nc.vector.tensor_tensor(
                            resl[:, s], rsl[:, s], E[:, s], Alu.mult)
                    else:
                        nc.vector.tensor_tensor(
                            b3(rsl), b3(rsl), b3(resl), Alu.subtract)
                        nc.vector.tensor_tensor(
                            b3(resl), b3(rsl), b3(E), Alu.mult)
                    for q in range(lo, hi):
                        nc.scalar.activation(
                            blk(ste, q), blk(resl, q), Act.Copy,
                            scale=float(tau[q]))
                # Stre tree first so the R1 update (and next iter's exps)
                # can start while the other trees run
                tree_sum(Stre, ste, nc.vector, "c")
                t = scratch.tile([P, FD], f32, tag="t")
                nc.vector.scalar_tensor_tensor(
                    t[:], Stre[:], -c1, B[:], Alu.mult, Alu.mult)
                nc.vector.tensor_add(R1[:], R1[:], t[:])
                nc.vector.tensor_scalar(
                    R1[:], R1[:], 0.0, LIM_R, Alu.max, Alu.min)
                tree_sum(Sr, rsl, nc.vector, "a")
                nc.vector.scalar_tensor_tensor(
                    A[:], Sr[:], -c1, A[:], Alu.mult, Alu.add)
                nc.vector.tensor_scalar(
                    A[:], A[:], 0.0, LIM_A, Alu.max, Alu.min)
                tree_sum(Sre, resl, nc.vector, "b")
                nc.vector.scalar_tensor_tensor(
                    B[:], Sre[:], c1, B[:], Alu.mult, Alu.add)
                nc.vector.tensor_scalar(
                    B[:], B[:], 0.0, LIM_B, Alu.max, Alu.min)
                if i in snap_iters:
                    snapshot(snap_idx)
                    snap_idx += 1
                continue
            else:
                # exps in two halves so DVE can start on the first half
                for q in range(NQ):
                    nc.scalar.activation(
                        blk(E, q), R1[:], Act.Exp, scale=-float(tau[q]))
                # r = (A - b) - B*e  built in two half-slab passes
                for lo, hi in ((0, 6), (6, NQ)):
                    s = slice(lo * FD, hi * FD)
                    nq = hi - lo
                    def b3(x):
                        return x[:, s].rearrange("p (q f) -> p q f", q=nq)
                    Abc = A[:].unsqueeze(1).broadcast_to([P, nq, FD])
                    Bbc = B[:].unsqueeze(1).broadcast_to([P, nq, FD])
                    nc.vector.tensor_tensor(
                        b3(rsl), Abc, b3(bsb), Alu.subtract)
                    nc.vector.tensor_tensor(
                        b3(resl), Bbc, b3(E), Alu.mult)
                    if variant == "v13":
                        # all-contiguous operands: flat 2-D APs
                        nc.vector.tensor_tensor(
                            rsl[:, s], rsl[:, s], resl[:, s], Alu.subtract)
                        nc.vector.tensor_tensor(
                            resl[:, s], rsl[:, s], E[:, s], Alu.mult)
                    else:
                        nc.vector.tensor_tensor(
                            b3(rsl), b3(rsl), b3(resl), Alu.subtract)
                        nc.vector.tensor_tensor(
                            b3(resl), b3(rsl), b3(E), Alu.mult)
                if variant == "v3":
                    # tau-weighted slab on ACT; two trees on GpSimd, one on DVE
                    for q in range(NQ):
                        nc.scalar.activation(
                            blk(ste, q), blk(resl, q), Act.Copy,
                            scale=float(tau[q]))
                    tree_sum(Sr, rsl, nc.gpsimd, "a")
                    tree_sum(Sre, resl, nc.gpsimd, "b")
                    tree_sum(Stre, ste, nc.vector, "c")
                elif variant == "v4":
                    # tau-weighted slab on ACT; all three trees on DVE
                    for q in range(NQ):
                        nc.scalar.activation(
                            blk(ste, q), blk(resl, q), Act.Copy,
                            scale=float(tau[q]))
                    tree_sum(Stre, ste, nc.vector, "c")
                    tree_sum(Sr, rsl, nc.vector, "a")
                    tree_sum(Sre, resl, nc.vector, "b")
                else:
                    tree_sum(Sr, rsl, nc.vector, "a")
                    tree_sum(Sre, resl, nc.vector, "b")
                    nc.vector.tensor_scalar_mul(
                        Stre[:], resl[:, 0:FD], float(tau[0]))
                    for q in range(1, NQ):
                        nc.vector.scalar_tensor_tensor(
                            Stre[:], blk(resl, q), float(tau[q]), Stre[:],
                            Alu.mult, Alu.add)
            # ---- update (R1 first so next iteration's exps start early) ----
            t = scratch.tile([P, FD], f32, tag="t")
            nc.vector.scalar_tensor_tensor(
                t[:], Stre[:], -c1, B[:], Alu.mult, Alu.mult)
            nc.vector.tensor_add(R1[:], R1[:], t[:])
            nc.vector.tensor_scalar(R1[:], R1[:], 0.0, LIM_R, Alu.max, Alu.min)
            nc.vector.scalar_tensor_tensor(
                A[:], Sr[:], -c1, A[:], Alu.mult, Alu.add)
            nc.vector.tensor_scalar(A[:], A[:], 0.0, LIM_A, Alu.max, Alu.min)
            nc.vector.scalar_tensor_tensor(
                B[:], Sre[:], c1, B[:], Alu.mult, Alu.add)
            nc.vector.tensor_scalar(B[:], B[:], 0.0, LIM_B, Alu.max, Alu.min)

            if i in snap_iters:
                snapshot(snap_idx)
                snap_idx += 1
    nc.compile()
    return nc


def _prep_core_input(b_img, compute_fp16=False):
    # b_img: [192,192,11] -> [128, 11*288] with pixel p = part*FD + col
    x = np.asarray(b_img, np.float32).reshape(NPIX, NQ)
    x = x.reshape(P, FD, NQ).transpose(0, 2, 1).reshape(P, NQ * FD)
    return np.ascontiguousarray(x.astype(np.float16 if compute_fp16 else np.float32))


def _unpack_out(o):
    # o: [n_snaps, 128, 3*288] -> [n_snaps, 192, 192, 3]
    n = o.shape[0]
    x = o.reshape(n, P, 3, FD).transpose(0, 1, 3, 2).reshape(n, 192, 192, 3)
    return x


def kernel(b, tau, mu, lm, W1, b1, Wm, bm, Wl, bl):
    from concourse.bass_utils import run_bass_kernel_spmd

    tau0 = [float(v) for v in np.asarray(tau, np.float32)[0]]
    mus = [float(v) for v in np.asarray(mu, np.float32)]
    lrs = [2.0] * N_GD + mus
    snap_iters = set(range(N_GD - 1, N_GD + NS))  # after iters 299..309

    nc = _build_nc(tau0, lrs, snap_iters, variant=VARIANT)

    in_maps = [{"b": _prep_core_input(b[c])} for c in range(N_CORES)]
    res = run_bass_kernel_spmd(nc, in_maps, core_ids=list(range(N_CORES)))
    outs = [_unpack_out(res.results[c]["out"]) for c in range(N_CORES)]
    full = np.stack(outs, axis=1)  # [12, 8, 192, 192, 3]
    return full.astype(np.float32)



# revision 3
# speedup vs baseline: 3.6033x; 3.6033x over previous
"""Trainium2 Bass kernel for the MOLLI T1-fitting unrolled-GD problem.

The reference computation reduces to (the denoiser CNN output is dead code,
and x - range_constraint(x) == 0 since x is always pre-clipped):

    x0 = clip([max_q|b|, 2*max_q|b|, 1], 0, [3,6,50])
    300 GD steps with lr=2.0, then 10 steps with lr=mu[i], where per pixel:
        e_q  = exp(-tau_q * R1)
        r_q  = A - B*e_q - b_q
        gA   = mean_q r_q ; gB = -mean_q r_q e_q ; gR1 = B*mean_q tau_q r_q e_q
        x    = clip(x - lr*g, 0, [3,6,50])
    output = stack([x0, x_after_300, ..., x_after_310])  # [12,8,192,192,3]

Sharding: pure data parallel, one batch image per NeuronCore (8 cores).
Per core: 36864 pixels laid out as SBUF [128 partitions, 288 free]; the
11-echo slab is [128, 11*288].

v14 design (vs the v11 all-DVE baseline at ~19.4us/iter):
  - fp16 for b and all slabs (DVE tensor_tensor 2x mode). Validated in
    numpy: maxerr ~2.3e-4 absolute vs tolerance 0.13.
  - state A/B/R1 kept in fp32 (fp16 state measured 0.16 err - fails);
    small fp16 shadow copies of A/B feed the slab products.
  - all three 11-block reduction trees moved from DVE to the TensorEngine
    as scaled-identity PSUM-accumulation chains (11 matmuls each, N=288).
    tau_q weights and the Sre sign fold into the stationaries for free,
    which also deletes v11's 11 ACT ste copies per iteration.
  - R1 chain + update emitted first so next iteration's ACT exps start
    while A/B updates still run.
"""

import numpy as np
from contextlib import ExitStack

NQ = 11
NPIX = 192 * 192          # 36864 pixels per image/core
P = 128                   # SBUF partitions
FD = NPIX // P            # 288 free elements per partition
N_GD = 300
NS = 10
N_CORES = 8
LIM_A, LIM_B, LIM_R = 3.0, 6.0, 50.0
VARIANT = "v14"


def _build_nc(tau, lrs, snap_iters, compute_fp16=True, variant=VARIANT):
    """Build the Bass program. tau: list of 11 floats. lrs: list of 310
    learning rates. snap_iters: iteration indices after which to snapshot."""
    import concourse.bass as bass
    import concourse.tile as tile
    import concourse.mybir as mybir
    from concourse import bacc

    f32 = mybir.dt.float32
    f16 = mybir.dt.float16
    Alu = mybir.AluOpType
    Act = mybir.ActivationFunctionType

    n_iters = len(lrs)
    n_snaps = 1 + len(snap_iters)

    nc = bacc.Bacc(trn_type="TRN2", target_bir_lowering=False, debug=False)
    b_d = nc.dram_tensor("b", [P, NQ * FD], f16, kind="ExternalInput").ap()
    # stationaries: [I | -I | tau_0*I ... tau_10*I] as one [128, 13*128] input
    w_d = nc.dram_tensor("wmat", [P, 13 * P], f16, kind="ExternalInput").ap()
    out_d = nc.dram_tensor(
        "out", [n_snaps, P, 3 * FD], f32, kind="ExternalOutput"
    ).ap()

    with ExitStack() as ctx:
        tc = ctx.enter_context(tile.TileContext(nc))
        state = ctx.enter_context(tc.tile_pool(name="state", bufs=1))
        slabs = ctx.enter_context(tc.tile_pool(name="slabs", bufs=2))
        scratch = ctx.enter_context(tc.tile_pool(name="scratch", bufs=2))
        psum = ctx.enter_context(tc.tile_pool(name="psum", bufs=2,
                                              space="PSUM"))

        bsb = state.tile([P, NQ * FD], f16, tag="bsb")
        wm = state.tile([P, 13 * P], f16, tag="wm")
        A = state.tile([P, FD], f32, tag="A")
        B = state.tile([P, FD], f32, tag="B")
        R1 = state.tile([P, FD], f32, tag="R1")
        A16 = state.tile([P, FD], f16, tag="A16")
        B16 = state.tile([P, FD], f16, tag="B16")

        WI = wm[:, 0:P]
        WnI = wm[:, P:2 * P]

        def Wtau(q):
            return wm[:, (2 + q) * P:(3 + q) * P]

        def blk(t, q):
            return t[:, q * FD:(q + 1) * FD]

        nc.sync.dma_start(bsb[:], b_d[:])
        nc.sync.dma_start(wm[:], w_d[:])

        # ---- init: A0 = max_q |b_q| ; A=clip(A0,0,3); B=min(2A0,6); R1=1 ----
        absb = scratch.tile([P, NQ * FD], f16, tag="absb")
        nc.scalar.activation(absb[:], bsb[:], Act.Abs)
        t1 = scratch.tile([P, 5 * FD], f16, tag="i5")
        nc.vector.tensor_tensor(
            t1[:], absb[:, 0:5 * FD], absb[:, 5 * FD:10 * FD], Alu.max)
        t2 = scratch.tile([P, 2 * FD], f16, tag="i2")
        nc.vector.tensor_tensor(
            t2[:], t1[:, 0:2 * FD], t1[:, 2 * FD:4 * FD], Alu.max)
        t3 = scratch.tile([P, FD], f16, tag="i1")
        nc.vector.tensor_tensor(t3[:], t2[:, 0:FD], t2[:, FD:2 * FD], Alu.max)
        nc.vector.tensor_tensor(t3[:], t3[:], t1[:, 4 * FD:5 * FD], Alu.max)
        nc.vector.tensor_tensor(t3[:], t3[:], blk(absb, 10), Alu.max)
        nc.vector.tensor_scalar(A[:], t3[:], 0.0, LIM_A, Alu.max, Alu.min)
        nc.vector.tensor_scalar(B[:], t3[:], 2.0, LIM_B, Alu.mult, Alu.min)
        nc.vector.memset(R1[:], 1.0)
        nc.vector.tensor_copy(A16[:], A[:])
        nc.vector.tensor_copy(B16[:], B[:])

        def snapshot(s):
            nc.sync.dma_start(out_d[s, :, 0:FD], A[:])
            nc.sync.dma_start(out_d[s, :, FD:2 * FD], B[:])
            nc.sync.dma_start(out_d[s, :, 2 * FD:3 * FD], R1[:])

        snapshot(0)

        snap_idx = 1
        for i in range(n_iters):
            c1 = float(lrs[i]) / NQ

            E = slabs.tile([P, NQ * FD], f16, tag="E", name="E")
            rsl = slabs.tile([P, NQ * FD], f16, tag="rsl", name="rsl")
            resl = slabs.tile([P, NQ * FD], f16, tag="resl", name="resl")
            ps_stre = psum.tile([P, 512], f32, tag="ps_stre", name="pstre")
            ps_sr = psum.tile([P, 512], f32, tag="ps_sr", name="psr")
            ps_nsre = psum.tile([P, 512], f32, tag="ps_nsre", name="pnsre")

            # q-chunked: DVE products of chunk 1 overlap ACT exps of chunk 2;
            # PE chains trail the products per chunk.
            for (lo, hi) in ((0, 6), (6, NQ)):
                for q in range(lo, hi):
                    nc.scalar.activation(
                        blk(E, q), R1[:], Act.Exp, scale=-float(tau[q]))
                s = slice(lo * FD, hi * FD)
                nq = hi - lo
                Abc = A16[:].unsqueeze(1).broadcast_to([P, nq, FD])
                Bbc = B16[:].unsqueeze(1).broadcast_to([P, nq, FD])

                def v3(x):
                    return x[:, s].rearrange("p (q f) -> p q f", q=nq)

                nc.vector.tensor_tensor(v3(rsl), Abc, v3(bsb), Alu.subtract)
                nc.vector.tensor_tensor(v3(resl), Bbc, v3(E), Alu.mult)
                nc.vector.tensor_tensor(
                    rsl[:, s], rsl[:, s], resl[:, s], Alu.subtract)
                nc.vector.tensor_tensor(
                    resl[:, s], rsl[:, s], E[:, s], Alu.mult)
                # chains: Stre emitted first so its last MM retires earliest
                for q in range(lo, hi):
                    nc.tensor.matmul(ps_stre[:, 0:FD], Wtau(q), blk(resl, q),
                                     start=(q == 0), stop=(q == NQ - 1))
                for q in range(lo, hi):
                    nc.tensor.matmul(ps_sr[:, 0:FD], WI, blk(rsl, q),
                                     start=(q == 0), stop=(q == NQ - 1))
                for q in range(lo, hi):
                    nc.tensor.matmul(ps_nsre[:, 0:FD], WnI, blk(resl, q),
                                     start=(q == 0), stop=(q == NQ - 1))

            # R1 first: unblocks next iteration's exps
            t = scratch.tile([P, FD], f32, tag="t", name="t")
            nc.vector.scalar_tensor_tensor(
                t[:], ps_stre[:, 0:FD], -c1, B[:], Alu.mult, Alu.mult)
            nc.vector.tensor_tensor(R1[:], R1[:], t[:], Alu.add)
            nc.vector.tensor_scalar(
                R1[:], R1[:], 0.0, LIM_R, Alu.max, Alu.min)

            nc.vector.scalar_tensor_tensor(
                A[:], ps_sr[:, 0:FD], -c1, A[:], Alu.mult, Alu.add)
            nc.vector.tensor_scalar(A[:], A[:], 0.0, LIM_A, Alu.max, Alu.min)
            nc.vector.tensor_copy(A16[:], A[:])

            nc.vector.scalar_tensor_tensor(
                B[:], ps_nsre[:, 0:FD], -c1, B[:], Alu.mult, Alu.add)
            nc.vector.tensor_scalar(B[:], B[:], 0.0, LIM_B, Alu.max, Alu.min)
            nc.vector.tensor_copy(B16[:], B[:])

            if i in snap_iters:
                snapshot(snap_idx)
                snap_idx += 1
    nc.compile()
    return nc


def _make_wmat(tau):
    """[128, 13*128] fp16: I | -I | tau_q * I for q=0..10."""
    I = np.eye(P, dtype=np.float32)
    mats = [I, -I] + [float(t) * I for t in tau]
    return np.concatenate(mats, axis=1).astype(np.float16)


def _prep_core_input(b_img):
    # b_img: [192,192,11] -> [128, 11*288] with pixel p = part*FD + col
    x = np.asarray(b_img, np.float32).reshape(NPIX, NQ)
    x = x.reshape(P, FD, NQ).transpose(0, 2, 1).reshape(P, NQ * FD)
    return np.ascontiguousarray(x.astype(np.float16))


def _unpack_out(o):
    # o: [n_snaps, 128, 3*288] -> [n_snaps, 192, 192, 3]
    n = o.shape[0]
    x = o.reshape(n, P, 3, FD).transpose(0, 1, 3, 2).reshape(n, 192, 192, 3)
    return x


def kernel(b, tau, mu, lm, W1, b1, Wm, bm, Wl, bl):
    from concourse.bass_utils import run_bass_kernel_spmd

    tau0 = [float(v) for v in np.asarray(tau, np.float32)[0]]
    mus = [float(v) for v in np.asarray(mu, np.float32)]
    lrs = [2.0] * N_GD + mus
    snap_iters = set(range(N_GD - 1, N_GD + NS))  # after iters 299..309

    nc = _build_nc(tau0, lrs, snap_iters)

    wmat = _make_wmat(tau0)
    in_maps = [{"b": _prep_core_input(b[c]), "wmat": wmat}
               for c in range(N_CORES)]
    res = run_bass_kernel_spmd(nc, in_maps, core_ids=list(range(N_CORES)))
    outs = [_unpack_out(res.results[c]["out"]) for c in range(N_CORES)]
    full = np.stack(outs, axis=1)  # [12, 8, 192, 192, 3]
    return full.astype(np.float32)
